# revision 1
# baseline (speedup 1.0000x reference)
"""Trainium2 Bass kernel for nn_ANN_Comp_29240137351521 (dense_cnn).

Reference computes, per batch row b of x [16384, 512] (complex, given as
real/imag f32 pairs):
    h = x @ w0                      # [B, 512] complex
    a = ifft(fft(h, n=1023)^2)      # full self-convolution, [B, 1023]
    out = |a @ wlast|               # [B, 10] f32

Algebraic collapse used here: the self-convolution + final contraction is a
polynomial-evaluation identity. With L = 1024 >= 2*512-1 evaluation points at
the L-th roots of unity:
    e   = x @ F        where F  = fft(w0, n=L, axis=1)        [512, L]
    z   = (e*e) @ Wt   where Wt = ifft(pad(wlast, L), axis=0) [L, 10]
    out = |z|
so the whole network is two dense matmuls + an elementwise complex square --
no FFT on device. F and Wt are tiny weight transforms folded on the host.

Real-expanded form computed on device (per core, data-parallel over batch),
using a Gauss 3-multiplication split of the complex matmul (12 real
matmuls per tile instead of 16; the PE stream is the binding resource):
    P1 = xr@Fr ; P2 = xi@Fi ; P3 = (xr+xi)@(Fr+Fi)    (PSUM accumulation)
    m = er-ei = 2*P1-P3 ;  p = er+ei = P3-2*P2        (DVE fused ops)
    sr = er^2-ei^2 = p*m ; p^2 ; m^2                  (ACT squares + DVE)
    [zr | zi] = sr@[Wtr|Wti] + (p^2-m^2)@[-Wti/2|Wtr/2]   (column-packed
                                                           second matmul)
    host: out = sqrt(zr^2 + zi^2)

Everything runs transposed (l on partitions, batch on the free axis) so the
second matmul needs no on-device transpose; x is fed pre-transposed from the
host in bf16 (measured end-to-end error 4e-3 of output scale, ~5x under the
2e-2 gate; squares/accumulations stay fp32). Weights and activations are
host-packed into [128, *] layouts so each input is one fat contiguous DMA on
a hardware DGE queue; dummy warm-up matmuls run during the load phase so the
PE HAM clock-gate is released before real work arrives.

Sharding: pure data parallel -- batch split 8 ways, weights replicated.
"""

import numpy as np
import ml_dtypes

import concourse.bass as bass
import concourse.mybir as mybir
from concourse import bacc, tile
from concourse.bass_utils import run_bass_kernel_spmd

NCORES = 8
B, D, L, C = 16384, 512, 1024, 10
BC = B // NCORES
P = 128
BN = 512
ND = D // P
NL = L // P
NB = BC // BN

F32 = mybir.dt.float32
BF16 = mybir.dt.bfloat16
ALU = mybir.AluOpType

_NC_CACHE = None


def build_nc():
    global _NC_CACHE
    if _NC_CACHE is not None:
        return _NC_CACHE

    nc = bacc.Bacc(None, target_bir_lowering=False)

    xtr_d = nc.declare_dram_parameter("xT_r", [P, ND, BC], BF16, isOutput=False)
    xti_d = nc.declare_dram_parameter("xT_i", [P, ND, BC], BF16, isOutput=False)
    xts_d = nc.declare_dram_parameter("xT_s", [P, ND, BC], BF16, isOutput=False)
    f1_d = nc.declare_dram_parameter("F_1", [P, ND * L], BF16, isOutput=False)
    f2_d = nc.declare_dram_parameter("F_2", [P, ND * L], BF16, isOutput=False)
    f3_d = nc.declare_dram_parameter("F_3", [P, ND * L], BF16, isOutput=False)
    wa_d = nc.declare_dram_parameter("WtA", [P, NL * 2 * C], BF16, isOutput=False)
    wp_d = nc.declare_dram_parameter("WtP", [P, NL * 2 * C], BF16, isOutput=False)
    wm_d = nc.declare_dram_parameter("WtM", [P, NL * 2 * C], BF16, isOutput=False)
    out_d = nc.declare_dram_parameter("out", [8 * C, BC], F32, isOutput=True)

    with tile.TileContext(nc) as tc:
        with (
            tc.tile_pool(name="wts", bufs=1) as wts,
            tc.tile_pool(name="xs", bufs=1) as xs,
            tc.tile_pool(name="tmp", bufs=3) as tmp,
            tc.tile_pool(name="sq", bufs=3) as sq,
            tc.tile_pool(name="zo", bufs=2) as zo,
            tc.tile_pool(name="pse", bufs=2, space="PSUM") as pse,
            tc.tile_pool(name="psz", bufs=2, space="PSUM") as psz,
        ):
            # PE warm-up during the load phase
            dummy = wts.tile([P, 64], BF16, tag="dummy")
            nc.gpsimd.memset(dummy[:], 0.0)
            wacc = pse.tile([64, 64], F32, tag="p1")
            for i in range(40):
                nc.tensor.matmul(wacc[:], dummy[:, 0:64], dummy[:],
                                 start=(i == 0), stop=False,
                                 skip_group_check=True)

            def warm_fill(n):
                for _ in range(n):
                    nc.tensor.matmul(wacc[:], dummy[:, 0:64], dummy[:],
                                     start=False, stop=False,
                                     skip_group_check=True)

            f1 = wts.tile([P, ND * L], BF16, tag="f1")
            f2 = wts.tile([P, ND * L], BF16, tag="f2")
            f3 = wts.tile([P, ND * L], BF16, tag="f3")
            xtr = xs.tile([P, ND, BC], BF16, tag="xtr")
            xti = xs.tile([P, ND, BC], BF16, tag="xti")
            xts = xs.tile([P, ND, BC], BF16, tag="xts")
            wa = wts.tile([P, NL * 2 * C], BF16, tag="wa")
            wp = wts.tile([P, NL * 2 * C], BF16, tag="wp")
            wm = wts.tile([P, NL * 2 * C], BF16, tag="wm")

            def fpair(k):       # two l-chunks of F (l-major): 256KB per DMA
                return slice(k * 2 * D, (k + 1) * 2 * D)

            # DMA order: tiny z-weights first (they'd otherwise block the
            # first z-batch), then F in l-pair chunks just ahead of use,
            # x(b0) as one 3D-AP DMA per stream, rest-x likewise. Few fat
            # dispatches -- the ~700ns per-dispatch cost paces the queues.
            nc.sync.dma_start(wa[:], wa_d[:])
            nc.sync.dma_start(wp[:], wp_d[:])
            nc.scalar.dma_start(wm[:], wm_d[:])
            nc.sync.dma_start(f1[:, fpair(0)], f1_d[:, fpair(0)])
            nc.scalar.dma_start(f2[:, fpair(0)], f2_d[:, fpair(0)])
            nc.sync.dma_start(xtr[:, :, 0:BN], xtr_d[:, :, 0:BN])
            nc.sync.dma_start(f3[:, fpair(0)], f3_d[:, fpair(0)])
            nc.scalar.dma_start(xti[:, :, 0:BN], xti_d[:, :, 0:BN])
            nc.scalar.dma_start(xts[:, :, 0:BN], xts_d[:, :, 0:BN])
            for k in range(1, ND):
                nc.sync.dma_start(f1[:, fpair(k)], f1_d[:, fpair(k)])
                nc.scalar.dma_start(f2[:, fpair(k)], f2_d[:, fpair(k)])
                nc.sync.dma_start(f3[:, fpair(k)], f3_d[:, fpair(k)])
            nc.sync.dma_start(xtr[:, :, BN:], xtr_d[:, :, BN:])
            nc.scalar.dma_start(xti[:, :, BN:], xti_d[:, :, BN:])
            nc.scalar.dma_start(xts[:, :, BN:], xts_d[:, :, BN:])

            def fsl(d, l):      # F weight chunk (d, l) in l-major packing
                return slice(l * D + d * P, l * D + (d + 1) * P)

            def wsl(l):
                return slice(l * 2 * C, (l + 1) * 2 * C)

            # z-matmuls run late (pending) so the PE never waits on DVE;
            # entries carry their zz tile, col-group and out slice.
            pending = []   # (zz, key, j, wt, wslice, rhs, bs)
            zcnt = {}      # (bkey, j) -> completed matmul count

            def zflush(batch):
                done = []
                for (pzz, key, j, wt, ws, rhs, bs) in batch:
                    n = zcnt.get((key, j), 0)
                    zcnt[(key, j)] = n + 1
                    stop = (n == NL * 3 // 4 - 1)
                    nc.tensor.matmul(
                        pzz[32 * j:32 * j + 2 * C, :], wt[:, ws], rhs[:],
                        start=(n == 0), stop=stop,
                        tile_position=(0, 32 * j), skip_group_check=True)
                    if stop:
                        done.append((pzz, j, bs))
                for (pzz, j, bs) in done:
                    zt = zo.tile([2 * C, BN], F32, tag=f"zt{j}")
                    nc.scalar.copy(zt[:], pzz[32 * j:32 * j + 2 * C, :])
                    nc.sync.dma_start(
                        out_d[2 * C * j:2 * C * (j + 1), bs], zt[:])

            warm_fill(50)
            gidx = 0
            for b in range(NB):
                bs = slice(b * BN, (b + 1) * BN)
                zz = psz.tile([P, BN], F32, tag="zz")
                for l in range(NL):
                    if b == 0 and l < 3:
                        warm_fill(8)
                    p1 = pse.tile([P, BN], F32, tag="p1")
                    p2 = pse.tile([P, BN], F32, tag="p2")
                    p3 = pse.tile([P, BN], F32, tag="p3")
                    for d in range(ND):
                        nc.tensor.matmul(
                            p1[:], f1[:, fsl(d, l)], xtr[:, d, bs],
                            start=(d == 0), stop=(d == ND - 1),
                            skip_group_check=True)
                    for d in range(ND):
                        nc.tensor.matmul(
                            p2[:], f2[:, fsl(d, l)], xti[:, d, bs],
                            start=(d == 0), stop=(d == ND - 1),
                            skip_group_check=True)
                    for d in range(ND):
                        nc.tensor.matmul(
                            p3[:], f3[:, fsl(d, l)], xts[:, d, bs],
                            start=(d == 0), stop=(d == ND - 1),
                            skip_group_check=True)

                    if len(pending) >= (4 if b == NB - 1 else 8):
                        zflush(pending[:4])
                        pending = pending[4:]

                    # c3 = P3 ; m = 2*P1 - c3 ; p = c3 - 2*P2  (f32)
                    # a = p^2 ; bq = m^2 ; sr = p*m            (bf16 out)
                    c3 = tmp.tile([P, BN], F32, tag="c3")
                    nc.scalar.copy(c3[:], p3[:])
                    m = tmp.tile([P, BN], F32, tag="m")
                    nc.vector.scalar_tensor_tensor(
                        m[:], p1[:], 2.0, c3[:], ALU.mult, ALU.subtract)
                    p = tmp.tile([P, BN], F32, tag="p")
                    nc.vector.scalar_tensor_tensor(
                        p[:], p2[:], -2.0, c3[:], ALU.mult, ALU.add)
                    a = sq.tile([P, BN], BF16, tag="a")
                    nc.scalar.square(a[:], p[:])
                    bq = sq.tile([P, BN], BF16, tag="bq")
                    nc.scalar.square(bq[:], m[:])
                    sr = sq.tile([P, BN], BF16, tag="sr")
                    nc.vector.tensor_mul(sr[:], p[:], m[:])

                    for wt, rhs in ((wa, sr), (wp, a), (wm, bq)):
                        pending.append((zz, b, gidx % 4, wt, wsl(l), rhs, bs))
                        gidx += 1

            while pending:
                zflush(pending[:4])
                pending = pending[4:]

    nc.compile()
    _NC_CACHE = nc
    return nc


def _pack128(arr):
    R = arr.shape[0] // P
    return np.ascontiguousarray(
        arr.reshape(R, P, arr.shape[1]).transpose(1, 0, 2).reshape(P, -1))


def _packF(a):
    """[512, 1024] -> [128, 4096] l-major: col l*512 + d*128 + c holds
    F[d*128+p, l*128+c], so one l-chunk's 4 contraction slices are
    contiguous and can be DMA'd just ahead of their first use."""
    return np.ascontiguousarray(
        a.reshape(ND, P, NL, P).transpose(1, 2, 0, 3).reshape(P, -1))


def _host_weights(w0_real, w0_imag, wlast_real, wlast_imag):
    w0 = w0_real.astype(np.float64) + 1j * w0_imag.astype(np.float64)
    wl = wlast_real.astype(np.float64) + 1j * wlast_imag.astype(np.float64)
    F = np.fft.fft(w0, n=L, axis=1)
    Wt = np.fft.ifft(
        np.concatenate([wl, np.zeros((1, C))], axis=0), axis=0)
    bf = ml_dtypes.bfloat16
    F1 = _packF(F.real.astype(bf))
    F2 = _packF(F.imag.astype(bf))
    F3 = _packF((F.real + F.imag).astype(bf))
    Wtr, Wti = Wt.real, Wt.imag
    WtA = _pack128(np.hstack([Wtr, Wti]).astype(bf))
    wb4 = np.hstack([-2 * Wti, 2 * Wtr]) / 4.0
    WtP = _pack128(wb4.astype(bf))
    WtM = _pack128((-wb4).astype(bf))
    return F1, F2, F3, WtA, WtP, WtM


def make_in_maps(x_real, x_imag, w0_real, w0_imag, wlast_real, wlast_imag):
    F1, F2, F3, WtA, WtP, WtM = _host_weights(
        w0_real, w0_imag, wlast_real, wlast_imag)
    bf = ml_dtypes.bfloat16
    xr = np.ascontiguousarray(x_real.T, dtype=bf)
    xi = np.ascontiguousarray(x_imag.T, dtype=bf)
    xsum = np.ascontiguousarray(
        (x_real.astype(np.float32) + x_imag.astype(np.float32)).T, dtype=bf)
    def pack3d(a):      # [512, BC] -> [128, 4, BC]
        return np.ascontiguousarray(
            a.reshape(ND, P, a.shape[1]).transpose(1, 0, 2))

    in_maps = []
    for c in range(NCORES):
        sl = slice(c * BC, (c + 1) * BC)
        in_maps.append({
            "xT_r": pack3d(xr[:, sl]),
            "xT_i": pack3d(xi[:, sl]),
            "xT_s": pack3d(xsum[:, sl]),
            "F_1": F1, "F_2": F2, "F_3": F3,
            "WtA": WtA, "WtP": WtP, "WtM": WtM,
        })
    return in_maps


def postprocess(results):
    outs = []
    for c in range(NCORES):
        o = results[c]["out"]
        z = o.reshape(4, 2 * C, BC).sum(axis=0)
        mag = np.sqrt(z[:C] ** 2 + z[C:2 * C] ** 2).T
        outs.append(mag)
    return np.ascontiguousarray(np.concatenate(outs, axis=0), dtype=np.float32)


def kernel(x_real, x_imag, w0_real, w0_imag, wlast_real, wlast_imag):
    x_real, x_imag, w0_real, w0_imag, wlast_real, wlast_imag = (
        np.asarray(arr) for arr in
        (x_real, x_imag, w0_real, w0_imag, wlast_real, wlast_imag))
    nc = build_nc()
    in_maps = make_in_maps(
        x_real, x_imag, w0_real, w0_imag, wlast_real, wlast_imag)
    # A stale/wedged NeuronCore (e.g. a previously killed process that died
    # mid-execute) fails with NRT_EXEC_UNIT_UNRECOVERABLE; reloading resets
    # it but may need a fresh backend and a moment. Retry a few times.
    import time
    last = None
    for attempt in range(4):
        try:
            res = run_bass_kernel_spmd(
                nc, in_maps, core_ids=list(range(NCORES)))
            return postprocess(res.results)
        except Exception as e:
            last = e
            time.sleep(2.0 + 2.0 * attempt)
            try:
                import jax
                import jax.extend.backend
                jax.clear_caches()
                jax.extend.backend.clear_backends()
            except Exception:
                pass
    raise last

